# revision 17
# baseline (speedup 1.0000x reference)
"""Trainium2 Bass kernel for a 2-layer GAT (GATConv x2) on 8 NeuronCores.

Strategy (dst-sharded graph parallelism):
- Nodes are partitioned across 8 cores (6250 each); every edge lives on the
  core owning its destination, grouped by 128-node dst-block, padded to a
  uniform subblock count so one SPMD program serves all cores.
- Layer 1 exploits linearity: h[src], alpha_src[src], alpha_dst[dst] are all
  linear in x, so the host uploads pre-gathered x[src]/x[dst] (transposed,
  stacked [128, L]) and ONE matmul per 128-edge subblock reproduces
  [h_src | as_src | ad_dst] rows on device - no device gathers at all.
- Segment softmax/aggregation use a selection-matrix matmul: S_T[e, r] =
  (dstoff_e == r), built with one DVE is_equal against a constant iota tile;
  PSUM accumulates S_T.T @ [exp*h | exp] over a block's subblocks. Softmax
  max-subtraction is skipped (logits are O(1), exp is safe) and the
  normalization divides after aggregation.
- Layer 2 (nonlinear input) gathers [h2|as2|ad2] 256B rows with the custom
  GPSIMD dma_gather (int16 indices -> two 25000-row table views; edges are
  grouped per block into A/B src-halves). ad2[dst] is expanded per edge with
  a tensor_tensor_reduce against a partition-replicated ad2 row. The small
  per-core layer-2 table is AllGather-ed between phases.
"""
import math
import numpy as np

N = 50000
F = 64
H1, C1 = 8, 8
D1 = H1 * C1          # 64
NCLS = 16
E = 800000
NEG = 0.2
M = 8                 # cores
NPC = N // M          # 6250
P = 128
NB = math.ceil(NPC / P)   # 49 dst blocks per core (last partial: 106 rows)
HALF = N // 2             # 25000 (int16 table split)

fp32 = None
i16 = None
SIM_SAFE = False
_LAST_NC = None
_LAST_INMAPS = None


def _host_prep(x, src, dst, W1, a_src1, a_dst1):
    """Group edges by (core, dst-block, src-half); pad to uniform subblock
    counts; build per-core upload arrays."""
    core = dst // NPC
    per_core = []
    # first pass: per-(core, block) A/B counts
    nA = np.zeros((M, NB), np.int64)
    nB = np.zeros((M, NB), np.int64)
    idx_by_core = []
    for k in range(M):
        m = np.nonzero(core == k)[0]
        s_k, d_k = src[m], dst[m]
        dloc = d_k - k * NPC
        blk = dloc // P
        half = (s_k >= HALF).astype(np.int64)
        np.add.at(nA[k], blk[half == 0], 1)
        np.add.at(nB[k], blk[half == 1], 1)
        idx_by_core.append((s_k, dloc, blk, half))
    SBA = int(math.ceil(nA.max() / P))
    SBB = int(math.ceil(nB.max() / P))
    SB2 = SBA + SBB
    L = NB * SB2 * P
    NSB = NB * SB2
    for k in range(M):
        s_k, dloc, blk, half = idx_by_core[k]
        # slot assignment: block b occupies subblocks [b*SB2, (b+1)*SB2);
        # A-edges fill subblocks [0, SBA) of the block, B-edges [SBA, SB2).
        srcs = np.zeros(L, np.int64)          # global src per slot (pad: 0)
        doff = np.full(L, -1.0, np.float32)   # dst offset in block (pad: -1)
        idx16v = np.zeros(L, np.int64)        # table row (A: s, B: s-HALF)
        orderA = np.lexsort((s_k, half, blk))
        fillA = np.zeros((NB,), np.int64)
        fillB = np.zeros((NB,), np.int64)
        for e in orderA:
            b = blk[e]
            if half[e] == 0:
                slot = b * SB2 * P + fillA[b]
                fillA[b] += 1
            else:
                slot = b * SB2 * P + SBA * P + fillB[b]
                fillB[b] += 1
            srcs[slot] = s_k[e]
            doff[slot] = dloc[e] - b * P
            idx16v[slot] = s_k[e] if half[e] == 0 else s_k[e] - HALF
        # dummy edges for pad rows of the partial last block, so every psum
        # row has a nonzero softmax denominator (no inf/NaN downstream)
        rows_last = NPC - (NB - 1) * P
        for r in range(rows_last, P):
            slot = (NB - 1) * SB2 * P + fillA[NB - 1]
            fillA[NB - 1] += 1
            srcs[slot] = 0
            doff[slot] = r
            idx16v[slot] = 0
        # upload arrays
        xs = x[srcs]                               # [L, F]
        dsts = (doff.astype(np.int64).clip(0) + (np.arange(L) // (SB2 * P)) * P
                + k * NPC)
        dsts[doff < 0] = 0
        np.clip(dsts, 0, x.shape[0] - 1, out=dsts)  # dummy pad-row edges
        xd = x[dsts]                               # [L, F]
        xsd = np.concatenate([xs, xd], axis=1).T.astype(np.float32)  # [128, L]
        xsd = np.ascontiguousarray(xsd)
        dstoff_T = np.ascontiguousarray(
            doff.reshape(NSB, P).T.astype(np.float32))     # [128, NSB]
        # int16 idx arrays, 16-wrapped per subblock, replicated to 128 parts
        iv = idx16v.reshape(NSB, P)                        # [NSB, 128]
        w = np.zeros((NSB, 16, 8), np.int16)
        ii = np.arange(P)
        w[:, ii % 16, ii // 16] = iv.astype(np.int16)
        idx16 = np.tile(w, (1, 8, 1)).reshape(NSB, 128, 8)
        idx16 = np.ascontiguousarray(idx16.transpose(1, 0, 2)
                                     .reshape(128, NSB * 8))
        per_core.append(dict(xsd=xsd, dstoff=dstoff_T, idx16=idx16))
    return per_core, SBA, SBB, SB2, NSB, L


def _build_program(SBA, SB2, NSB, L, nq=4, sim_safe=False, dbg=None):
    dbg = dbg or set()
    import concourse.bacc as bacc
    import concourse.bass as bass
    import concourse.mybir as mybir
    import concourse.tile as tile
    from concourse.library_config import mlp as mlp_lib

    fp32 = mybir.dt.float32
    i16 = mybir.dt.int16
    AF = mybir.ActivationFunctionType
    OP = mybir.AluOpType

    nc = bacc.Bacc("TRN2", target_bir_lowering=False, debug=False,
                   num_devices=M, num_swdge_queues=nq)
    xsd_in = nc.dram_tensor("xsd", [P, L], fp32, kind="ExternalInput")
    dstoff_in = nc.dram_tensor("dstoff", [P, NSB], fp32, kind="ExternalInput")
    idx_in = nc.dram_tensor("idx16", [P, NSB * 8], i16, kind="ExternalInput")
    WA_in = nc.dram_tensor("WA", [P, 72], fp32, kind="ExternalInput")
    W2_in = nc.dram_tensor("W2", [D1, NCLS], fp32, kind="ExternalInput")
    ASD2_in = nc.dram_tensor("ASD2", [NCLS, 2], fp32, kind="ExternalInput")
    IDENT_in = nc.dram_tensor("IDENT", [P, P], fp32, kind="ExternalInput")
    IOTA_in = nc.dram_tensor("IOTA", [P, P], fp32, kind="ExternalInput")
    b1_in = nc.dram_tensor("b1", [1, D1], fp32, kind="ExternalInput")
    b2_in = nc.dram_tensor("b2", [1, NCLS], fp32, kind="ExternalInput")
    out_ext = nc.dram_tensor("out", [NPC, NCLS], fp32, kind="ExternalOutput")

    with tile.TileContext(nc) as tc:
        nc.gpsimd.load_library(mlp_lib)
        with (
            tc.tile_pool(name="const", bufs=1) as cp,
            tc.tile_pool(name="sbuf", bufs=3) as sb,
            tc.tile_pool(name="psum", bufs=1, space="PSUM") as pp,
            tc.tile_pool(name="psag", bufs=2, space="PSUM") as pag,
            tc.tile_pool(name="dram", bufs=1, space="DRAM") as dr,
        ):
            # constants
            WA = cp.tile([P, 72], fp32)
            nc.sync.dma_start(out=WA[:], in_=WA_in.ap())
            W2 = cp.tile([D1, NCLS], fp32)
            nc.sync.dma_start(out=W2[:], in_=W2_in.ap())
            ASD2 = cp.tile([NCLS, 2], fp32)
            nc.sync.dma_start(out=ASD2[:], in_=ASD2_in.ap())
            IDENT = cp.tile([P, P], fp32)
            nc.sync.dma_start(out=IDENT[:], in_=IDENT_in.ap())
            IOTA = cp.tile([P, P], fp32)
            nc.sync.dma_start(out=IOTA[:], in_=IOTA_in.ap())
            b1rep = cp.tile([P, D1], fp32)
            nc.sync.dma_start(out=b1rep[:], in_=b1_in.ap().to_broadcast((P, D1)))
            b2rep = cp.tile([P, NCLS], fp32)
            nc.sync.dma_start(out=b2rep[:], in_=b2_in.ap().to_broadcast((P, NCLS)))
            dstoffT = cp.tile([P, NSB], fp32)
            nc.sync.dma_start(out=dstoffT[:], in_=dstoff_in.ap())
            idx16 = cp.tile([P, NSB * 8], i16)
            nc.sync.dma_start(out=idx16[:], in_=idx_in.ap())
            ad2row = cp.tile([1, NB * P], fp32)
            nc.gpsimd.memset(ad2row[:], 0.0)
            zeros64 = cp.tile([P, 64], fp32)
            nc.gpsimd.memset(zeros64[:], 0.0)
            ad2rep = cp.tile([P, NB * P], fp32)

            T2part = dr.tile([NPC, 64], fp32)
            T2full = dr.tile([N, 64], fp32)

            # ---------------- layer 1 + layer-2 node prep ----------------
            for b in range(NB):
                rows = min(P, NPC - b * P)
                agg = pag.tile([P, 72], fp32, tag="agg")
                for s in range(SB2):
                    sbk = b * SB2 + s
                    xsd = sb.tile([P, P], fp32, tag="xsd")
                    nc.sync.dma_start(
                        out=xsd[:], in_=xsd_in.ap()[:, sbk * P:(sbk + 1) * P])
                    g1 = pag.tile([P, 72], fp32, tag="g1")
                    nc.tensor.matmul(out=g1[:], lhsT=xsd[:], rhs=WA[:],
                                     start=True, stop=True)
                    lr = sb.tile([P, H1], fp32, tag="lr")
                    lr0 = sb.tile([P, H1], fp32, tag="lr0")
                    nc.vector.tensor_scalar_mul(out=lr0[:], in0=g1[:, 64:72],
                                                scalar1=NEG)
                    nc.vector.tensor_tensor(out=lr[:], in0=g1[:, 64:72],
                                            in1=lr0[:], op=OP.max)
                    ex = sb.tile([P, H1], fp32, tag="ex")
                    nc.scalar.activation(ex[:], lr[:], AF.Exp)
                    m_t = sb.tile([P, 72], fp32, tag="m")
                    nc.vector.tensor_tensor(
                        out=m_t[:, 0:64].rearrange("p (h c) -> p h c", h=H1),
                        in0=g1[:, 0:64].rearrange("p (h c) -> p h c", h=H1),
                        in1=ex[:, :, None].to_broadcast((P, H1, C1)),
                        op=OP.mult)
                    nc.vector.tensor_copy(m_t[:, 64:72], ex[:])
                    s_t = sb.tile([P, P], fp32, tag="st")
                    nc.vector.tensor_tensor(
                        out=s_t[:],
                        in0=dstoffT[:, sbk:sbk + 1].to_broadcast((P, P)),
                        in1=IOTA[:], op=OP.is_equal)
                    nc.tensor.matmul(out=agg[:], lhsT=s_t[:], rhs=m_t[:],
                                     start=(s == 0), stop=(s == SB2 - 1))
                # finalize block: y = num/Z + b1, elu -> h1
                zt = sb.tile([P, H1], fp32, tag="zt")
                nc.vector.reciprocal(out=zt[:], in_=agg[:, 64:72])
                y = sb.tile([P, D1], fp32, tag="y")
                nc.vector.tensor_tensor(
                    out=y[:].rearrange("p (h c) -> p h c", h=H1),
                    in0=agg[:, 0:64].rearrange("p (h c) -> p h c", h=H1),
                    in1=zt[:, :, None].to_broadcast((P, H1, C1)),
                    op=OP.mult)
                t0 = sb.tile([P, D1], fp32, tag="t0")
                nc.vector.tensor_add(out=t0[:], in0=y[:], in1=b1rep[:])
                tm = sb.tile([P, D1], fp32, tag="tm")
                nc.vector.tensor_scalar_min(out=tm[:], in0=t0[:], scalar1=0.0)
                u = sb.tile([P, D1], fp32, tag="u")
                nc.scalar.activation(u[:], tm[:], AF.Exp)
                r = sb.tile([P, D1], fp32, tag="r")
                nc.scalar.activation(r[:], t0[:], AF.Relu)
                v = sb.tile([P, D1], fp32, tag="v")
                nc.vector.tensor_scalar(out=v[:], in0=u[:], scalar1=1.0,
                                        scalar2=1.0, op0=OP.min,
                                        op1=OP.subtract)
                h1 = sb.tile([P, D1], fp32, tag="h1")
                nc.vector.tensor_add(out=h1[:], in0=r[:], in1=v[:])
                # layer-2 node prep: h2 = h1 @ W2, as2/ad2 = h2 @ a2
                h1T_p = pp.tile([D1, P], fp32, tag="t1")
                nc.tensor.transpose(out=h1T_p[:], in_=h1[:], identity=IDENT[:])
                h1T = sb.tile([D1, P], fp32, tag="h1T")
                nc.vector.tensor_copy(h1T[:], h1T_p[:])
                h2T_p = pp.tile([NCLS, P], fp32, tag="t2")
                nc.tensor.matmul(out=h2T_p[:], lhsT=W2[:], rhs=h1T[:],
                                 start=True, stop=True)
                h2T = sb.tile([NCLS, P], fp32, tag="h2T")
                nc.vector.tensor_copy(h2T[:], h2T_p[:])
                a2T_p = pp.tile([2, P], fp32, tag="t3")
                nc.tensor.matmul(out=a2T_p[:], lhsT=ASD2[:], rhs=h2T[:],
                                 start=True, stop=True)
                a2T = sb.tile([2, P], fp32, tag="a2T")
                nc.vector.tensor_copy(a2T[:], a2T_p[:])
                h2_p = pp.tile([P, NCLS], fp32, tag="t1")
                nc.tensor.transpose(out=h2_p[:], in_=h2T[:],
                                    identity=IDENT[:NCLS, :NCLS])
                a2_p = pp.tile([P, 2], fp32, tag="t2")
                nc.tensor.transpose(out=a2_p[:], in_=a2T[:],
                                    identity=IDENT[:2, :2])
                t2t = sb.tile([P, 64], fp32, tag="t2t")
                nc.vector.tensor_copy(t2t[:, NCLS + 1:], zeros64[:, NCLS + 1:])
                nc.vector.tensor_copy(t2t[:, 0:NCLS], h2_p[:])
                nc.vector.tensor_copy(t2t[:, NCLS:NCLS + 1], a2_p[:, 1:2])
                nc.sync.dma_start(out=T2part[b * P:b * P + rows, :],
                                  in_=t2t[:rows])
                nc.vector.tensor_copy(ad2row[:, b * P:b * P + rows],
                                      a2T[0:1, :rows])

            # ---------------- exchange ----------------
            nc.gpsimd.partition_broadcast(out_ap=ad2rep[:], in_ap=ad2row[:],
                                          channels=P)
            nc.gpsimd.collective_compute(
                "AllGather", mybir.AluOpType.bypass,
                replica_groups=[list(range(M))],
                ins=[T2part.opt()], outs=[T2full.opt()])

            # ---------------- layer 2 ----------------
            tblA = T2full[0:HALF, :]
            tblB = T2full[HALF:N, :]
            for b in ([] if "skip_l2" in dbg else range(NB)):
                rows = min(P, NPC - b * P)
                agg2 = pag.tile([P, 17], fp32, tag="agg")
                for s in range(SB2):
                    sbk = b * SB2 + s
                    g2 = sb.tile([P, 64], fp32, tag="g2")
                    if "no_gather" in dbg:
                        nc.sync.dma_start(out=g2[:], in_=T2full[0:P, :])
                    else:
                        nc.gpsimd.dma_gather(
                            out_ap=g2[:].rearrange("p (g d) -> p g d", g=1),
                            in_ap=(tblA if s < SBA else tblB),
                            idxs_ap=idx16[:, sbk * 8:(sbk + 1) * 8],
                            num_idxs=P, num_idxs_reg=P, elem_size=64,
                            queue_num=s % nq)
                    s_t = sb.tile([P, P], fp32, tag="st")
                    nc.vector.tensor_tensor(
                        out=s_t[:],
                        in0=dstoffT[:, sbk:sbk + 1].to_broadcast((P, P)),
                        in1=IOTA[:], op=OP.is_equal)
                    scr = sb.tile([P, P], fp32, tag="scr")
                    scr2 = sb.tile([P, P], fp32, tag="scr2")
                    ad2e = sb.tile([P, 1], fp32, tag="ad2e")
                    nc.vector.tensor_tensor(
                        out=scr[:], in0=s_t[:],
                        in1=ad2rep[:, b * P:(b + 1) * P], op=OP.mult)
                    nc.scalar.activation(scr2[:], scr[:], AF.Copy,
                                         accum_out=ad2e[:])
                    e2 = sb.tile([P, 1], fp32, tag="e2")
                    nc.vector.tensor_tensor(out=e2[:], in0=g2[:, 16:17],
                                            in1=ad2e[:], op=OP.add)
                    lr2 = sb.tile([P, 1], fp32, tag="lr2")
                    lr20 = sb.tile([P, 1], fp32, tag="lr20")
                    nc.vector.tensor_scalar_mul(out=lr20[:], in0=e2[:],
                                                scalar1=NEG)
                    nc.vector.tensor_tensor(out=lr2[:], in0=e2[:],
                                            in1=lr20[:], op=OP.max)
                    ex2 = sb.tile([P, 1], fp32, tag="ex2")
                    nc.scalar.activation(ex2[:], lr2[:], AF.Exp)
                    m2 = sb.tile([P, 17], fp32, tag="m2")
                    nc.vector.tensor_tensor(
                        out=m2[:, 0:NCLS], in0=g2[:, 0:NCLS],
                        in1=ex2[:].to_broadcast((P, NCLS)), op=OP.mult)
                    nc.vector.tensor_copy(m2[:, NCLS:NCLS + 1], ex2[:])
                    nc.tensor.matmul(out=agg2[:], lhsT=s_t[:], rhs=m2[:],
                                     start=(s == 0), stop=(s == SB2 - 1))
                # finalize: logits = num/Z + b2, log_softmax
                z2 = sb.tile([P, 1], fp32, tag="z2")
                nc.vector.reciprocal(out=z2[:], in_=agg2[:, NCLS:NCLS + 1])
                lg = sb.tile([P, NCLS], fp32, tag="lg")
                nc.vector.tensor_tensor(
                    out=lg[:], in0=agg2[:, 0:NCLS],
                    in1=z2[:].to_broadcast((P, NCLS)),
                    op=OP.mult)
                lgb = sb.tile([P, NCLS], fp32, tag="lgb")
                nc.vector.tensor_add(out=lgb[:], in0=lg[:], in1=b2rep[:])
                et = sb.tile([P, NCLS], fp32, tag="et")
                se = sb.tile([P, 1], fp32, tag="se")
                nc.scalar.activation(et[:], lgb[:], AF.Exp, accum_out=se[:])
                lse = sb.tile([P, 1], fp32, tag="lse")
                nc.scalar.activation(lse[:], se[:], AF.Ln)
                o_t = sb.tile([P, NCLS], fp32, tag="o")
                nc.vector.tensor_tensor(
                    out=o_t[:], in0=lgb[:],
                    in1=lse[:].to_broadcast((P, NCLS)), op=OP.subtract)
                nc.sync.dma_start(out=out_ext[b * P:b * P + rows, :],
                                  in_=o_t[:rows])
    nc.compile()
    return nc


def kernel(x, edge_index, W1, a_src1, a_dst1, b1, W2, a_src2, a_dst2, b2):
    x = np.asarray(x, np.float32)
    edge_index = np.asarray(edge_index)
    W1 = np.asarray(W1, np.float32)
    a_src1 = np.asarray(a_src1, np.float32)
    a_dst1 = np.asarray(a_dst1, np.float32)
    b1 = np.asarray(b1, np.float32)
    W2 = np.asarray(W2, np.float32)
    a_src2 = np.asarray(a_src2, np.float32)
    a_dst2 = np.asarray(a_dst2, np.float32)
    b2 = np.asarray(b2, np.float32)

    loop = np.arange(N, dtype=np.int64)
    src = np.concatenate([edge_index[0].astype(np.int64), loop])
    dst = np.concatenate([edge_index[1].astype(np.int64), loop])

    per_core, SBA, SBB, SB2, NSB, L = _host_prep(x, src, dst, W1, a_src1,
                                                 a_dst1)

    # WA: [x_src | x_dst] (128) -> [h_src(64) | as_src(8) | ad_dst(8)]
    W1r = W1.reshape(F, H1, C1)
    As_x = np.einsum("fhc,hc->fh", W1r, a_src1)     # [64, 8]
    Ad_x = np.einsum("fhc,hc->fh", W1r, a_dst1)     # [64, 8]
    WA = np.zeros((P, 72), np.float32)
    WA[0:F, 0:64] = W1
    WA[0:F, 64:72] = As_x
    WA[F:2 * F, 64:72] = Ad_x
    # col 0 = a_dst2 (ad2 lands on partition 0 of a2T for the row copy),
    # col 1 = a_src2
    ASD2 = np.stack([a_dst2[0], a_src2[0]], axis=1).astype(np.float32)  # [16,2]
    IDENT = np.eye(P, dtype=np.float32)
    IOTA = np.broadcast_to(np.arange(P, dtype=np.float32), (P, P)).copy()

    nc = _build_program(SBA, SB2, NSB, L, sim_safe=SIM_SAFE)

    from concourse.bass_utils import run_bass_kernel_spmd
    in_maps = []
    for k in range(M):
        d = per_core[k]
        in_maps.append({
            "xsd": d["xsd"], "dstoff": d["dstoff"], "idx16": d["idx16"],
            "WA": WA, "W2": W2, "ASD2": ASD2, "IDENT": IDENT, "IOTA": IOTA,
            "b1": b1[None, :], "b2": b2[None, :],
        })
    global _LAST_NC, _LAST_INMAPS
    _LAST_NC, _LAST_INMAPS = nc, in_maps
    res = run_bass_kernel_spmd(nc, in_maps, list(range(M))).results
    out = np.concatenate([res[k]["out"] for k in range(M)], axis=0)
    return out.astype(np.float32)


# revision 18
# speedup vs baseline: 1.1469x; 1.1469x over previous
"""Trainium2 Bass kernel for a 2-layer GAT (GATConv x2) on 8 NeuronCores.

Strategy (dst-sharded graph parallelism):
- Nodes are partitioned across 8 cores (6250 each); every edge lives on the
  core owning its destination, grouped by 128-node dst-block, padded to a
  uniform subblock count so one SPMD program serves all cores.
- Layer 1 exploits linearity: h[src], alpha_src[src], alpha_dst[dst] are all
  linear in x, so the host uploads pre-gathered x[src]/x[dst] (transposed,
  stacked [128, L]) and ONE matmul per 128-edge subblock reproduces
  [h_src | as_src | ad_dst] rows on device - no device gathers at all.
- Segment softmax/aggregation use a selection-matrix matmul: S_T[e, r] =
  (dstoff_e == r), built with one DVE is_equal against a constant iota tile;
  PSUM accumulates S_T.T @ [exp*h | exp] over a block's subblocks. Softmax
  max-subtraction is skipped (logits are O(1), exp is safe) and the
  normalization divides after aggregation.
- Layer 2 (nonlinear input) gathers [h2|as2|ad2] 256B rows with the custom
  GPSIMD dma_gather (int16 indices -> two 25000-row table views; edges are
  grouped per block into A/B src-halves). ad2[dst] is expanded per edge with
  a tensor_tensor_reduce against a partition-replicated ad2 row. The small
  per-core layer-2 table is AllGather-ed between phases.
"""
import math
import numpy as np

N = 50000
F = 64
H1, C1 = 8, 8
D1 = H1 * C1          # 64
NCLS = 16
E = 800000
NEG = 0.2
M = 8                 # cores
NPC = N // M          # 6250
P = 128
NB = math.ceil(NPC / P)   # 49 dst blocks per core (last partial: 106 rows)
HALF = N // 2             # 25000 (int16 table split)

fp32 = None
i16 = None
SIM_SAFE = False
_LAST_NC = None
_LAST_INMAPS = None


def _host_prep(x, src, dst, W1, a_src1, a_dst1):
    """Group edges by (core, dst-block, src-half); pad to uniform subblock
    counts; build per-core upload arrays."""
    core = dst // NPC
    per_core = []
    # first pass: per-(core, block) A/B counts
    nA = np.zeros((M, NB), np.int64)
    nB = np.zeros((M, NB), np.int64)
    idx_by_core = []
    for k in range(M):
        m = np.nonzero(core == k)[0]
        s_k, d_k = src[m], dst[m]
        dloc = d_k - k * NPC
        blk = dloc // P
        half = (s_k >= HALF).astype(np.int64)
        np.add.at(nA[k], blk[half == 0], 1)
        np.add.at(nB[k], blk[half == 1], 1)
        idx_by_core.append((s_k, dloc, blk, half))
    SBA = int(math.ceil(nA.max() / P))
    SBB = int(math.ceil(nB.max() / P))
    SB2 = SBA + SBB
    L = NB * SB2 * P
    NSB = NB * SB2
    for k in range(M):
        s_k, dloc, blk, half = idx_by_core[k]
        # slot assignment: block b occupies subblocks [b*SB2, (b+1)*SB2);
        # A-edges fill subblocks [0, SBA) of the block, B-edges [SBA, SB2).
        srcs = np.zeros(L, np.int64)          # global src per slot (pad: 0)
        doff = np.full(L, -1.0, np.float32)   # dst offset in block (pad: -1)
        idx16v = np.zeros(L, np.int64)        # table row (A: s, B: s-HALF)
        orderA = np.lexsort((s_k, half, blk))
        fillA = np.zeros((NB,), np.int64)
        fillB = np.zeros((NB,), np.int64)
        for e in orderA:
            b = blk[e]
            if half[e] == 0:
                slot = b * SB2 * P + fillA[b]
                fillA[b] += 1
            else:
                slot = b * SB2 * P + SBA * P + fillB[b]
                fillB[b] += 1
            srcs[slot] = s_k[e]
            doff[slot] = dloc[e] - b * P
            idx16v[slot] = s_k[e] if half[e] == 0 else s_k[e] - HALF
        # dummy edges for pad rows of the partial last block, so every psum
        # row has a nonzero softmax denominator (no inf/NaN downstream)
        rows_last = NPC - (NB - 1) * P
        for r in range(rows_last, P):
            slot = (NB - 1) * SB2 * P + fillA[NB - 1]
            fillA[NB - 1] += 1
            srcs[slot] = 0
            doff[slot] = r
            idx16v[slot] = 0
        # upload arrays
        xs = x[srcs]                               # [L, F]
        dsts = (doff.astype(np.int64).clip(0) + (np.arange(L) // (SB2 * P)) * P
                + k * NPC)
        dsts[doff < 0] = 0
        np.clip(dsts, 0, x.shape[0] - 1, out=dsts)  # dummy pad-row edges
        xd = x[dsts]                               # [L, F]
        xsd = np.concatenate([xs, xd], axis=1).T.astype(np.float32)  # [128, L]
        xsd = np.ascontiguousarray(xsd)
        dstoff_T = np.ascontiguousarray(
            doff.reshape(NSB, P).T.astype(np.float32))     # [128, NSB]
        # int16 idx arrays, 16-wrapped per subblock, replicated to 128 parts
        iv = idx16v.reshape(NSB, P)                        # [NSB, 128]
        w = np.zeros((NSB, 16, 8), np.int16)
        ii = np.arange(P)
        w[:, ii % 16, ii // 16] = iv.astype(np.int16)
        idx16 = np.tile(w, (1, 8, 1)).reshape(NSB, 128, 8)
        idx16 = np.ascontiguousarray(idx16.transpose(1, 0, 2)
                                     .reshape(128, NSB * 8))
        per_core.append(dict(xsd=xsd, dstoff=dstoff_T, idx16=idx16))
    return per_core, SBA, SBB, SB2, NSB, L


def _build_program(SBA, SB2, NSB, L, nq=4, sim_safe=False, dbg=None):
    dbg = dbg or set()
    import concourse.bacc as bacc
    import concourse.bass as bass
    import concourse.mybir as mybir
    import concourse.tile as tile
    from concourse.library_config import mlp as mlp_lib

    fp32 = mybir.dt.float32
    bf16 = mybir.dt.bfloat16
    i16 = mybir.dt.int16
    AF = mybir.ActivationFunctionType
    OP = mybir.AluOpType

    nc = bacc.Bacc("TRN2", target_bir_lowering=False, debug=False,
                   num_devices=M, num_swdge_queues=nq)
    xsd_in = nc.dram_tensor("xsd", [P, L], fp32, kind="ExternalInput")
    dstoff_in = nc.dram_tensor("dstoff", [P, NSB], fp32, kind="ExternalInput")
    idx_in = nc.dram_tensor("idx16", [P, NSB * 8], i16, kind="ExternalInput")
    WA_in = nc.dram_tensor("WA", [P, 72], fp32, kind="ExternalInput")
    W2_in = nc.dram_tensor("W2", [D1, NCLS], fp32, kind="ExternalInput")
    ASD2_in = nc.dram_tensor("ASD2", [NCLS, 2], fp32, kind="ExternalInput")
    IDENT_in = nc.dram_tensor("IDENT", [P, P], fp32, kind="ExternalInput")
    IOTA_in = nc.dram_tensor("IOTA", [P, P], fp32, kind="ExternalInput")
    b1_in = nc.dram_tensor("b1", [1, D1], fp32, kind="ExternalInput")
    b2_in = nc.dram_tensor("b2", [1, NCLS], fp32, kind="ExternalInput")
    out_ext = nc.dram_tensor("out", [NPC, NCLS], fp32, kind="ExternalOutput")

    with tile.TileContext(nc) as tc:
        nc.gpsimd.load_library(mlp_lib)
        with (
            tc.tile_pool(name="const", bufs=1) as cp,
            tc.tile_pool(name="sbuf", bufs=3) as sb,
            tc.tile_pool(name="psum", bufs=1, space="PSUM") as pp,
            tc.tile_pool(name="psag", bufs=2, space="PSUM") as pag,
            tc.tile_pool(name="dram", bufs=1, space="DRAM") as dr,
        ):
            # constants
            WA = cp.tile([P, 72], fp32)
            nc.sync.dma_start(out=WA[:], in_=WA_in.ap())
            W2 = cp.tile([D1, NCLS], fp32)
            nc.sync.dma_start(out=W2[:], in_=W2_in.ap())
            ASD2 = cp.tile([NCLS, 2], fp32)
            nc.sync.dma_start(out=ASD2[:], in_=ASD2_in.ap())
            IDENT = cp.tile([P, P], fp32)
            nc.sync.dma_start(out=IDENT[:], in_=IDENT_in.ap())
            IOTA = cp.tile([P, P], fp32)
            nc.sync.dma_start(out=IOTA[:], in_=IOTA_in.ap())
            b1rep = cp.tile([P, D1], fp32)
            nc.sync.dma_start(out=b1rep[:], in_=b1_in.ap().to_broadcast((P, D1)))
            b2rep = cp.tile([P, NCLS], fp32)
            nc.sync.dma_start(out=b2rep[:], in_=b2_in.ap().to_broadcast((P, NCLS)))
            dstoffT = cp.tile([P, NSB], fp32)
            nc.sync.dma_start(out=dstoffT[:], in_=dstoff_in.ap())
            idx16 = cp.tile([P, NSB * 8], i16)
            nc.sync.dma_start(out=idx16[:], in_=idx_in.ap())
            ad2row = cp.tile([1, NB * P], fp32)
            nc.gpsimd.memset(ad2row[:], 0.0)
            zeros64 = cp.tile([P, 64], fp32)
            nc.gpsimd.memset(zeros64[:], 0.0)
            ad2rep = cp.tile([P, NB * P], fp32)

            T2part = dr.tile([NPC, 64], fp32)
            T2full = dr.tile([N, 64], fp32)

            # ---------------- layer 1 + layer-2 node prep ----------------
            for b in range(NB):
                rows = min(P, NPC - b * P)
                agg = pag.tile([P, 72], fp32, tag="agg")
                for s in range(SB2):
                    sbk = b * SB2 + s
                    xsd = sb.tile([P, P], fp32, tag="xsd")
                    nc.sync.dma_start(
                        out=xsd[:], in_=xsd_in.ap()[:, sbk * P:(sbk + 1) * P])
                    g1 = pag.tile([P, 72], fp32, tag="g1")
                    nc.tensor.matmul(out=g1[:], lhsT=xsd[:], rhs=WA[:],
                                     start=True, stop=True)
                    lr = sb.tile([P, H1], fp32, tag="lr")
                    lr0 = sb.tile([P, H1], fp32, tag="lr0")
                    nc.vector.tensor_scalar_mul(out=lr0[:], in0=g1[:, 64:72],
                                                scalar1=NEG)
                    nc.vector.tensor_tensor(out=lr[:], in0=g1[:, 64:72],
                                            in1=lr0[:], op=OP.max)
                    ex = sb.tile([P, H1], fp32, tag="ex")
                    nc.scalar.activation(ex[:], lr[:], AF.Exp)
                    m_t = sb.tile([P, 72], bf16, tag="m")
                    nc.vector.tensor_tensor(
                        out=m_t[:, 0:64].rearrange("p (h c) -> p h c", h=H1),
                        in0=g1[:, 0:64].rearrange("p (h c) -> p h c", h=H1),
                        in1=ex[:, :, None].to_broadcast((P, H1, C1)),
                        op=OP.mult)
                    nc.vector.tensor_copy(m_t[:, 64:72], ex[:])
                    s_t = sb.tile([P, P], bf16, tag="st")
                    nc.vector.tensor_tensor(
                        out=s_t[:],
                        in0=dstoffT[:, sbk:sbk + 1].to_broadcast((P, P)),
                        in1=IOTA[:], op=OP.is_equal)
                    nc.tensor.matmul(out=agg[:], lhsT=s_t[:], rhs=m_t[:],
                                     start=(s == 0), stop=(s == SB2 - 1))
                # finalize block: y = num/Z + b1, elu -> h1
                zt = sb.tile([P, H1], fp32, tag="zt")
                nc.vector.reciprocal(out=zt[:], in_=agg[:, 64:72])
                y = sb.tile([P, D1], fp32, tag="y")
                nc.vector.tensor_tensor(
                    out=y[:].rearrange("p (h c) -> p h c", h=H1),
                    in0=agg[:, 0:64].rearrange("p (h c) -> p h c", h=H1),
                    in1=zt[:, :, None].to_broadcast((P, H1, C1)),
                    op=OP.mult)
                t0 = sb.tile([P, D1], fp32, tag="t0")
                nc.vector.tensor_add(out=t0[:], in0=y[:], in1=b1rep[:])
                tm = sb.tile([P, D1], fp32, tag="tm")
                nc.vector.tensor_scalar_min(out=tm[:], in0=t0[:], scalar1=0.0)
                u = sb.tile([P, D1], fp32, tag="u")
                nc.scalar.activation(u[:], tm[:], AF.Exp)
                r = sb.tile([P, D1], fp32, tag="r")
                nc.scalar.activation(r[:], t0[:], AF.Relu)
                v = sb.tile([P, D1], fp32, tag="v")
                nc.vector.tensor_scalar(out=v[:], in0=u[:], scalar1=1.0,
                                        scalar2=1.0, op0=OP.min,
                                        op1=OP.subtract)
                h1 = sb.tile([P, D1], fp32, tag="h1")
                nc.vector.tensor_add(out=h1[:], in0=r[:], in1=v[:])
                # layer-2 node prep: h2 = h1 @ W2, as2/ad2 = h2 @ a2
                h1T_p = pp.tile([D1, P], fp32, tag="t1")
                nc.tensor.transpose(out=h1T_p[:], in_=h1[:], identity=IDENT[:])
                h1T = sb.tile([D1, P], fp32, tag="h1T")
                nc.vector.tensor_copy(h1T[:], h1T_p[:])
                h2T_p = pp.tile([NCLS, P], fp32, tag="t2")
                nc.tensor.matmul(out=h2T_p[:], lhsT=W2[:], rhs=h1T[:],
                                 start=True, stop=True)
                h2T = sb.tile([NCLS, P], fp32, tag="h2T")
                nc.vector.tensor_copy(h2T[:], h2T_p[:])
                a2T_p = pp.tile([2, P], fp32, tag="t3")
                nc.tensor.matmul(out=a2T_p[:], lhsT=ASD2[:], rhs=h2T[:],
                                 start=True, stop=True)
                a2T = sb.tile([2, P], fp32, tag="a2T")
                nc.vector.tensor_copy(a2T[:], a2T_p[:])
                h2_p = pp.tile([P, NCLS], fp32, tag="t1")
                nc.tensor.transpose(out=h2_p[:], in_=h2T[:],
                                    identity=IDENT[:NCLS, :NCLS])
                a2_p = pp.tile([P, 2], fp32, tag="t2")
                nc.tensor.transpose(out=a2_p[:], in_=a2T[:],
                                    identity=IDENT[:2, :2])
                t2t = sb.tile([P, 64], fp32, tag="t2t")
                nc.vector.tensor_copy(t2t[:, NCLS + 1:], zeros64[:, NCLS + 1:])
                nc.vector.tensor_copy(t2t[:, 0:NCLS], h2_p[:])
                nc.vector.tensor_copy(t2t[:, NCLS:NCLS + 1], a2_p[:, 1:2])
                nc.sync.dma_start(out=T2part[b * P:b * P + rows, :],
                                  in_=t2t[:rows])
                nc.vector.tensor_copy(ad2row[:, b * P:b * P + rows],
                                      a2T[0:1, :rows])

            # ---------------- exchange ----------------
            nc.gpsimd.partition_broadcast(out_ap=ad2rep[:], in_ap=ad2row[:],
                                          channels=P)
            nc.gpsimd.collective_compute(
                "AllGather", mybir.AluOpType.bypass,
                replica_groups=[list(range(M))],
                ins=[T2part.opt()], outs=[T2full.opt()])

            # ---------------- layer 2 ----------------
            tblA = T2full[0:HALF, :]
            tblB = T2full[HALF:N, :]
            for b in ([] if "skip_l2" in dbg else range(NB)):
                rows = min(P, NPC - b * P)
                agg2 = pag.tile([P, 17], fp32, tag="agg")
                for s in range(SB2):
                    sbk = b * SB2 + s
                    g2 = sb.tile([P, 64], fp32, tag="g2")
                    if "no_gather" in dbg:
                        nc.sync.dma_start(out=g2[:], in_=T2full[0:P, :])
                    else:
                        nc.gpsimd.dma_gather(
                            out_ap=g2[:].rearrange("p (g d) -> p g d", g=1),
                            in_ap=(tblA if s < SBA else tblB),
                            idxs_ap=idx16[:, sbk * 8:(sbk + 1) * 8],
                            num_idxs=P, num_idxs_reg=P, elem_size=64,
                            queue_num=s % nq)
                    s_t = sb.tile([P, P], fp32, tag="st")
                    nc.vector.tensor_tensor(
                        out=s_t[:],
                        in0=dstoffT[:, sbk:sbk + 1].to_broadcast((P, P)),
                        in1=IOTA[:], op=OP.is_equal)
                    scr = sb.tile([P, P], fp32, tag="scr")
                    scr2 = sb.tile([P, P], fp32, tag="scr2")
                    ad2e = sb.tile([P, 1], fp32, tag="ad2e")
                    nc.vector.tensor_tensor(
                        out=scr[:], in0=s_t[:],
                        in1=ad2rep[:, b * P:(b + 1) * P], op=OP.mult)
                    nc.scalar.activation(scr2[:], scr[:], AF.Copy,
                                         accum_out=ad2e[:])
                    e2 = sb.tile([P, 1], fp32, tag="e2")
                    nc.vector.tensor_tensor(out=e2[:], in0=g2[:, 16:17],
                                            in1=ad2e[:], op=OP.add)
                    lr2 = sb.tile([P, 1], fp32, tag="lr2")
                    lr20 = sb.tile([P, 1], fp32, tag="lr20")
                    nc.vector.tensor_scalar_mul(out=lr20[:], in0=e2[:],
                                                scalar1=NEG)
                    nc.vector.tensor_tensor(out=lr2[:], in0=e2[:],
                                            in1=lr20[:], op=OP.max)
                    ex2 = sb.tile([P, 1], fp32, tag="ex2")
                    nc.scalar.activation(ex2[:], lr2[:], AF.Exp)
                    m2 = sb.tile([P, 17], fp32, tag="m2")
                    nc.vector.tensor_tensor(
                        out=m2[:, 0:NCLS], in0=g2[:, 0:NCLS],
                        in1=ex2[:].to_broadcast((P, NCLS)), op=OP.mult)
                    nc.vector.tensor_copy(m2[:, NCLS:NCLS + 1], ex2[:])
                    nc.tensor.matmul(out=agg2[:], lhsT=s_t[:], rhs=m2[:],
                                     start=(s == 0), stop=(s == SB2 - 1))
                # finalize: logits = num/Z + b2, log_softmax
                z2 = sb.tile([P, 1], fp32, tag="z2")
                nc.vector.reciprocal(out=z2[:], in_=agg2[:, NCLS:NCLS + 1])
                lg = sb.tile([P, NCLS], fp32, tag="lg")
                nc.vector.tensor_tensor(
                    out=lg[:], in0=agg2[:, 0:NCLS],
                    in1=z2[:].to_broadcast((P, NCLS)),
                    op=OP.mult)
                lgb = sb.tile([P, NCLS], fp32, tag="lgb")
                nc.vector.tensor_add(out=lgb[:], in0=lg[:], in1=b2rep[:])
                et = sb.tile([P, NCLS], fp32, tag="et")
                se = sb.tile([P, 1], fp32, tag="se")
                nc.scalar.activation(et[:], lgb[:], AF.Exp, accum_out=se[:])
                lse = sb.tile([P, 1], fp32, tag="lse")
                nc.scalar.activation(lse[:], se[:], AF.Ln)
                o_t = sb.tile([P, NCLS], fp32, tag="o")
                nc.vector.tensor_tensor(
                    out=o_t[:], in0=lgb[:],
                    in1=lse[:].to_broadcast((P, NCLS)), op=OP.subtract)
                nc.sync.dma_start(out=out_ext[b * P:b * P + rows, :],
                                  in_=o_t[:rows])
    nc.compile()
    return nc


def kernel(x, edge_index, W1, a_src1, a_dst1, b1, W2, a_src2, a_dst2, b2):
    x = np.asarray(x, np.float32)
    edge_index = np.asarray(edge_index)
    W1 = np.asarray(W1, np.float32)
    a_src1 = np.asarray(a_src1, np.float32)
    a_dst1 = np.asarray(a_dst1, np.float32)
    b1 = np.asarray(b1, np.float32)
    W2 = np.asarray(W2, np.float32)
    a_src2 = np.asarray(a_src2, np.float32)
    a_dst2 = np.asarray(a_dst2, np.float32)
    b2 = np.asarray(b2, np.float32)

    loop = np.arange(N, dtype=np.int64)
    src = np.concatenate([edge_index[0].astype(np.int64), loop])
    dst = np.concatenate([edge_index[1].astype(np.int64), loop])

    per_core, SBA, SBB, SB2, NSB, L = _host_prep(x, src, dst, W1, a_src1,
                                                 a_dst1)

    # WA: [x_src | x_dst] (128) -> [h_src(64) | as_src(8) | ad_dst(8)]
    W1r = W1.reshape(F, H1, C1)
    As_x = np.einsum("fhc,hc->fh", W1r, a_src1)     # [64, 8]
    Ad_x = np.einsum("fhc,hc->fh", W1r, a_dst1)     # [64, 8]
    WA = np.zeros((P, 72), np.float32)
    WA[0:F, 0:64] = W1
    WA[0:F, 64:72] = As_x
    WA[F:2 * F, 64:72] = Ad_x
    # col 0 = a_dst2 (ad2 lands on partition 0 of a2T for the row copy),
    # col 1 = a_src2
    ASD2 = np.stack([a_dst2[0], a_src2[0]], axis=1).astype(np.float32)  # [16,2]
    IDENT = np.eye(P, dtype=np.float32)
    IOTA = np.broadcast_to(np.arange(P, dtype=np.float32), (P, P)).copy()

    nc = _build_program(SBA, SB2, NSB, L, sim_safe=SIM_SAFE)

    from concourse.bass_utils import run_bass_kernel_spmd
    in_maps = []
    for k in range(M):
        d = per_core[k]
        in_maps.append({
            "xsd": d["xsd"], "dstoff": d["dstoff"], "idx16": d["idx16"],
            "WA": WA, "W2": W2, "ASD2": ASD2, "IDENT": IDENT, "IOTA": IOTA,
            "b1": b1[None, :], "b2": b2[None, :],
        })
    global _LAST_NC, _LAST_INMAPS
    _LAST_NC, _LAST_INMAPS = nc, in_maps
    res = run_bass_kernel_spmd(nc, in_maps, list(range(M))).results
    out = np.concatenate([res[k]["out"] for k in range(M)], axis=0)
    return out.astype(np.float32)
